# revision 2
# baseline (speedup 1.0000x reference)
"""HGNN 2-layer hetero GNN on 8 TRN2 NeuronCores via Bass/Tile — v2 hybrid.

v2 vs baseline:
  - Gather calls batched across windows: per relation the tile stream is laid
    out as chunks of consecutive windows, each chunk split into a lo-block
    (src < 32768) and hi-block so one dma_gather call covers many windows.
  - Each block's leading fraction F_IND of tiles is fetched with
    indirect_dma_start (async, ~0 Pool engine time) while dma_gather
    (Pool-bound, ~961ns/tile) covers the rest -> the two mechanisms overlap.
  - AllGathers issued as early as possible (cc/cn after c-side layer 1,
    nn after n-side layer 1) so they hide under gather work.
  - Slow tensor_scalar DVE epilogues replaced with tensor_tensor forms.
"""
import math
import numpy as np
import ml_dtypes

bf16 = ml_dtypes.bfloat16

N = 50000
D = 128
P = 128
NCORES = 8
ROWS = N // NCORES          # 6250 dst rows per core
NW = math.ceil(ROWS / P)    # 49 windows (48*128 + 106)
LAST_ROWS = ROWS - (NW - 1) * P  # 106
ROWSP = NW * P              # 6272
PAD_DOFF = 200.0            # never matches iota 0..127
HI = 32768                  # int16 index split point

CH_C = 6                    # windows per chunk, cc relation
CH_N = 4                    # windows per chunk, cn/nn relations
import os
F_IND = float(os.environ.get("K2_F_IND", "0.0"))                 # fraction of tiles per block routed to indirect
MAX_CALL_TILES = 32         # max tiles per dma_gather / indirect call

_cache = {}


# ----------------------------------------------------------------------------
# Host preprocessing
# ----------------------------------------------------------------------------

def _pack_relation(src, dst, chunk_windows):
    """Pack one relation's edges into a chunked lo/hi block tile stream.

    Layout: chunks of `chunk_windows` consecutive windows. Within a chunk:
    first all lo tiles (window-major), then all hi tiles. Tiles stay aligned
    to (window, half) groups (Tlo/Thi = max over cores), so the per-window
    doff/one-hot semantics match the baseline.

    Returns per-core idx16 [128, 8*S] (gather), idx32 [128, S] (indirect),
    doff [128, S] bf16, plus scheduling metadata.
    """
    src = np.asarray(src, dtype=np.int64)
    dst = np.asarray(dst, dtype=np.int64)
    is_hi = (src >= HI).astype(np.int64)
    core = dst // ROWS
    dloc = dst % ROWS
    w = dloc // P
    gid2 = (core * NW + w) * 2 + is_hi
    order = np.argsort(gid2, kind="stable")
    src_s = src[order]
    gid2_s = gid2[order]
    dloc_s = dloc[order]
    core_s = core[order]
    w_s = w[order]
    hi_s = is_hi[order]

    counts = np.bincount(gid2_s, minlength=NCORES * NW * 2)
    starts = np.concatenate([[0], np.cumsum(counts)[:-1]])
    rank = np.arange(len(src_s)) - starts[gid2_s]

    cw = counts.reshape(NCORES, NW, 2)
    Tlo = np.ceil(cw[:, :, 0] / P).astype(np.int64).max(axis=0)  # [NW]
    Thi = np.ceil(cw[:, :, 1] / P).astype(np.int64).max(axis=0)

    # chunk structure
    chunks = []          # list of dicts
    tile_of = {}         # (w, half) -> global tile index of group start
    S = 0
    w0 = 0
    while w0 < NW:
        w1 = min(w0 + chunk_windows, NW)
        base = S
        lo_off = {}
        off = 0
        for wi in range(w0, w1):
            lo_off[wi] = off
            off += int(Tlo[wi])
        lo_T = off
        hi_off = {}
        for wi in range(w0, w1):
            hi_off[wi] = off
            off += int(Thi[wi])
        tot_T = off
        for wi in range(w0, w1):
            tile_of[(wi, 0)] = base + lo_off[wi]
            tile_of[(wi, 1)] = base + hi_off[wi]
        chunks.append(dict(w0=w0, w1=w1, base=base, lo_T=lo_T,
                           hi_T=tot_T - lo_T, tot_T=tot_T,
                           lo_off=lo_off, hi_off=hi_off))
        S += tot_T
        w0 = w1

    # slot assignment: edge -> (global tile, partition)
    half_s = hi_s
    gstart = np.zeros((NW, 2), np.int64)
    for wi in range(NW):
        gstart[wi, 0] = tile_of[(wi, 0)] * P
        gstart[wi, 1] = tile_of[(wi, 1)] * P
    slot = gstart[w_s, half_s] + rank
    colG = slot // P
    pG = slot % P

    doff = np.full((NCORES, P, S), PAD_DOFF, dtype=np.float32)
    doff[core_s, pG, colG] = (dloc_s - w_s * P).astype(np.float32)

    idx16 = np.zeros((NCORES, 16, S * 8), dtype=np.int16)
    col16 = colG * 8 + (pG // 16)
    row16 = pG % 16
    val16 = np.where(half_s == 1, src_s - HI, src_s).astype(np.int16)
    idx16[core_s, row16, col16] = val16
    idx16_full = np.ascontiguousarray(np.tile(idx16, (1, 8, 1)))

    idx32 = np.zeros((NCORES, P, S), dtype=np.int32)
    idx32[core_s, pG, colG] = src_s.astype(np.int32)

    return dict(S=S, Tlo=[int(t) for t in Tlo], Thi=[int(t) for t in Thi],
                chunks=chunks, tile_of=tile_of,
                idx16=idx16_full, idx32=idx32, doff=doff.astype(bf16))


def _bcast_rows(vec):
    v = np.zeros(ROWSP, dtype=vec.dtype)
    v[:ROWS] = vec
    return np.ascontiguousarray(np.broadcast_to(v[None, :], (P, ROWSP)))


def _pack_part(vec):
    out = np.zeros((P, NW), dtype=np.float32)
    padded = np.zeros(NW * P, dtype=np.float32)
    padded[:ROWS] = vec
    out[:] = padded.reshape(NW, P).T
    return out


def _prep(inp):
    feat_C = np.asarray(inp["feat_C"], dtype=np.float32)
    feat_N = np.asarray(inp["feat_N"], dtype=np.float32)

    def deg(x, n):
        return np.bincount(np.asarray(x, dtype=np.int64), minlength=n).astype(np.float32)

    cc_src = np.asarray(inp["cc_src"]); cc_dst = np.asarray(inp["cc_dst"])
    cn_src = np.asarray(inp["cn_src"]); cn_dst = np.asarray(inp["cn_dst"])
    nn_src = np.asarray(inp["nn_src"]); nn_dst = np.asarray(inp["nn_dst"])

    ns_cc = np.maximum(deg(cc_src, N), 1.0) ** -0.5
    nd_cc = np.maximum(deg(cc_dst, N), 1.0) ** -0.5
    ns_cn = np.maximum(deg(cn_src, N), 1.0) ** -0.5
    nd_cn = np.maximum(deg(cn_dst, N), 1.0) ** -0.5
    invd_nn = 1.0 / np.maximum(deg(nn_dst, N), 1.0)

    featC_cc = (feat_C * ns_cc[:, None]).astype(bf16)
    featC_cn = (feat_C * ns_cn[:, None]).astype(bf16)
    featN_b = feat_N.astype(bf16)

    rel_cc = _pack_relation(cc_src, cc_dst, CH_C)
    rel_cn = _pack_relation(cn_src, cn_dst, CH_N)
    rel_nn = _pack_relation(nn_src, nn_dst, CH_N)

    iota_b = np.ascontiguousarray(
        np.broadcast_to(np.arange(P, dtype=np.float32)[None, :], (P, P))).astype(bf16)

    def bb(v):
        return np.ascontiguousarray(np.broadcast_to(
            np.asarray(v, np.float32)[None, :], (P, P)))

    b1N = np.asarray(inp["b1_cn"], np.float32) + np.asarray(inp["b1_nn"], np.float32)
    b2N = np.asarray(inp["b2_cn"], np.float32) + np.asarray(inp["b2_nn"], np.float32)

    Wn = ["w1cc", "w1cn", "w1self", "w1neigh", "w2cc", "w2cn", "w2self", "w2neigh"]
    Wv = [inp["W1_cc"], inp["W1_cn"], inp["W1_self"], inp["W1_neigh"],
          inp["W2_cc"], inp["W2_cn"], inp["W2_self"], inp["W2_neigh"]]

    in_maps = []
    for c in range(NCORES):
        r0, r1 = c * ROWS, (c + 1) * ROWS
        m = {
            "featC_cc": featC_cc,
            "featC_cn": featC_cn,
            "featN_b": featN_b,
            "featNT_s": np.ascontiguousarray(np.concatenate(
                [featN_b[r0:r1], np.zeros((ROWSP - ROWS, D), bf16)]).T),
            "ndcc_b": _bcast_rows(nd_cc[r0:r1]).astype(bf16),
            "ndcn_b": _bcast_rows(nd_cn[r0:r1]).astype(bf16),
            "invd_b": _bcast_rows(invd_nn[r0:r1]).astype(bf16),
            "nsrccc_p": _pack_part(ns_cc[r0:r1]),
            "nsrccn_p": _pack_part(ns_cn[r0:r1]),
            "iota_b": iota_b,
            "zeros_b": np.zeros((P, P), np.float32),
            "b1C_b": bb(inp["b1_cc"]), "b1N_b": bb(b1N),
            "b2C_b": bb(inp["b2_cc"]), "b2N_b": bb(b2N),
            "b1N_col": np.ascontiguousarray(b1N[:, None]),
        }
        for rel, dat in (("cc", rel_cc), ("cn", rel_cn), ("nn", rel_nn)):
            m[f"idx_{rel}"] = dat["idx16"][c]
            m[f"idx32_{rel}"] = dat["idx32"][c]
            m[f"doff_{rel}"] = dat["doff"][c]
        for nm, v in zip(Wn, Wv):
            m[nm] = np.asarray(v, np.float32).astype(bf16)
        in_maps.append(m)

    meta = {}
    for rel, dat in (("cc", rel_cc), ("cn", rel_cn), ("nn", rel_nn)):
        meta[rel] = {k: dat[k] for k in ("S", "Tlo", "Thi", "chunks", "tile_of")}
    return in_maps, meta


# ----------------------------------------------------------------------------
# Bass kernel builder
# ----------------------------------------------------------------------------

def _build(meta):
    import concourse.bass as bass
    import concourse.bacc as bacc
    import concourse.mybir as mybir
    import concourse.tile as tile

    f32 = mybir.dt.float32
    bf = mybir.dt.bfloat16
    i16 = mybir.dt.int16
    i32 = mybir.dt.int32
    AOP = mybir.AluOpType

    nc = bacc.Bacc(None, target_bir_lowering=False)

    ext = {}
    def din(name, shape, dtype):
        ext[name] = nc.dram_tensor(name, shape, dtype, kind="ExternalInput")
        return ext[name]

    din("featC_cc", [N, D], bf)
    din("featC_cn", [N, D], bf)
    din("featN_b", [N, D], bf)
    din("featNT_s", [P, ROWSP], bf)
    for rel in ("cc", "cn", "nn"):
        S = meta[rel]["S"]
        din(f"idx_{rel}", [P, S * 8], i16)
        din(f"idx32_{rel}", [P, S], i32)
        din(f"doff_{rel}", [P, S], bf)
    din("ndcc_b", [P, ROWSP], bf)
    din("ndcn_b", [P, ROWSP], bf)
    din("invd_b", [P, ROWSP], bf)
    din("nsrccc_p", [P, NW], f32)
    din("nsrccn_p", [P, NW], f32)
    din("iota_b", [P, P], bf)
    din("zeros_b", [P, P], f32)
    din("b1C_b", [P, P], f32)
    din("b1N_b", [P, P], f32)
    din("b2C_b", [P, P], f32)
    din("b2N_b", [P, P], f32)
    din("b1N_col", [P, 1], f32)
    for nm in ("w1cc", "w1cn", "w1self", "w1neigh", "w2cc", "w2cn", "w2self", "w2neigh"):
        din(nm, [D, D], bf)

    oC_s = nc.dram_tensor("oC_s", [ROWS, D], f32, kind="ExternalOutput")
    oN_s = nc.dram_tensor("oN_s", [ROWS, D], f32, kind="ExternalOutput")

    ag_in = {r: nc.dram_tensor(f"agin_{r}", [ROWS, D], bf) for r in ("cc", "cn", "nn")}
    ag_out = {r: nc.dram_tensor(f"agout_{r}", [N, D], bf, addr_space="Shared")
              for r in ("cc", "cn", "nn")}

    with tile.TileContext(nc) as tc:
        import contextlib
        with contextlib.ExitStack() as ctx:
            cpool = ctx.enter_context(tc.tile_pool(name="consts", bufs=1))
            mpool = ctx.enter_context(tc.tile_pool(name="mchunks", bufs=2))
            work = ctx.enter_context(tc.tile_pool(name="work", bufs=3))
            psum = ctx.enter_context(tc.tile_pool(name="psum", bufs=2, space="PSUM"))

            sb = {}
            def load(name, shape, dtype):
                t = cpool.tile(shape, dtype, name=f"sb_{name}")
                nc.sync.dma_start(out=t[:], in_=ext[name][:])
                sb[name] = t
                return t

            for rel in ("cc", "cn", "nn"):
                S = meta[rel]["S"]
                load(f"idx_{rel}", [P, S * 8], i16)
                load(f"idx32_{rel}", [P, S], i32)
                load(f"doff_{rel}", [P, S], bf)
            load("ndcc_b", [P, ROWSP], bf)
            load("ndcn_b", [P, ROWSP], bf)
            load("invd_b", [P, ROWSP], bf)
            load("nsrccc_p", [P, NW], f32)
            load("nsrccn_p", [P, NW], f32)
            load("iota_b", [P, P], bf)
            load("zeros_b", [P, P], f32)
            load("b1C_b", [P, P], f32)
            load("b1N_b", [P, P], f32)
            load("b2C_b", [P, P], f32)
            load("b2N_b", [P, P], f32)
            load("b1N_col", [P, 1], f32)
            load("featNT_s", [P, ROWSP], bf)
            for nm in ("w1cc", "w1cn", "w1self", "w1neigh",
                       "w2cc", "w2cn", "w2self", "w2neigh"):
                load(nm, [D, D], bf)

            hNT = cpool.tile([P, ROWSP], bf, name="hNT")

            def col_bcast(t, c):
                """[128,1] column c of tile t -> [128, P] broadcast AP."""
                col = t[:, c:c + 1]
                return bass.AP(col.tensor, col.offset, [col.ap[0], [0, P]])

            def fetch_chunk(rel, ch, srcs):
                """Gather one chunk's tiles into a fresh m buffer.

                Leading F_IND of each (lo, hi) block via indirect_dma_start,
                remainder via dma_gather (lo/hi idx16 with split tables).
                Returns the m tile [P, tot_T, P].
                """
                md = meta[rel]
                tot = ch["tot_T"]
                base = ch["base"]
                m = mpool.tile([P, tot, P], bf, tag=f"m_{rel}", name=f"m_{rel}_{ch['w0']}")
                idx32 = sb[f"idx32_{rel}"]
                idx16 = sb[f"idx_{rel}"]

                for half, blk_off, blk_T in ((0, 0, ch["lo_T"]),
                                             (1, ch["lo_T"], ch["hi_T"])):
                    if blk_T == 0:
                        continue
                    a = int(round(blk_T * F_IND))
                    # indirect leading part: HW supports one row per partition
                    # per call, so issue one call per 128-row tile (~30ns each
                    # on the Pool sequencer; DGE + transfer run async).
                    for t0 in range(a):
                        g0 = base + blk_off + t0
                        nc.gpsimd.indirect_dma_start(
                            out=m[:, blk_off + t0, :],
                            out_offset=None,
                            in_=srcs[:, :],
                            in_offset=bass.IndirectOffsetOnAxis(
                                ap=idx32[:, g0:g0 + 1], axis=0))
                    # gather rest
                    t0 = a
                    tab = srcs[:, :] if half == 0 else srcs[HI:, :]
                    while t0 < blk_T:
                        n = min(MAX_CALL_TILES, blk_T - t0)
                        g0 = base + blk_off + t0
                        nc.gpsimd.dma_gather(
                            m[:, blk_off + t0:blk_off + t0 + n, :], tab,
                            idx16[:, g0 * 8:(g0 + n) * 8],
                            n * P, n * P, P, single_packet=False)
                        t0 += n
                return m

            def window_agg(rel, ch, w, m, norm_sb, ptag):
                """One window's segment aggregate -> aggT [P(feat), P(dst)]."""
                md = meta[rel]
                Tlo, Thi = md["Tlo"][w], md["Thi"][w]
                T = Tlo + Thi
                if T == 0:
                    T = 1  # shouldn't happen; guard
                lo_c = ch["lo_off"][w]           # chunk-local col of lo tiles
                hi_c = ch["hi_off"][w]
                g_lo = ch["base"] + lo_c          # global doff col
                g_hi = ch["base"] + hi_c
                doff = sb[f"doff_{rel}"]
                io = sb["iota_b"][:]

                O = work.tile([P, T * P], bf, tag="O", name=f"O_{rel}_{w}")
                if Tlo:
                    d = doff[:, g_lo:g_lo + Tlo]
                    in0 = bass.AP(d.tensor, d.offset, d.ap + [[0, P]])
                    in1 = bass.AP(io.tensor, io.offset, [io.ap[0], [0, Tlo], io.ap[1]])
                    nc.vector.tensor_tensor(out=O[:, 0:Tlo * P], in0=in0, in1=in1,
                                            op=AOP.is_equal)
                if Thi:
                    d = doff[:, g_hi:g_hi + Thi]
                    in0 = bass.AP(d.tensor, d.offset, d.ap + [[0, P]])
                    in1 = bass.AP(io.tensor, io.offset, [io.ap[0], [0, Thi], io.ap[1]])
                    nc.vector.tensor_tensor(out=O[:, Tlo * P:T * P], in0=in0, in1=in1,
                                            op=AOP.is_equal)

                pA = psum.tile([P, P], f32, tag=ptag, name=f"pA_{rel}_{w}")
                k = 0
                for t in range(Tlo):
                    nc.tensor.matmul(pA[:], lhsT=m[:, lo_c + t, :],
                                     rhs=O[:, k * P:(k + 1) * P],
                                     start=(k == 0), stop=(k == T - 1))
                    k += 1
                for t in range(Thi):
                    nc.tensor.matmul(pA[:], lhsT=m[:, hi_c + t, :],
                                     rhs=O[:, k * P:(k + 1) * P],
                                     start=(k == 0), stop=(k == T - 1))
                    k += 1
                aggT = work.tile([P, P], bf, tag=f"aggT_{rel}", name=f"aggT_{rel}_{w}")
                nc.vector.tensor_mul(aggT[:], pA[:], norm_sb[:, w * P:w * P + P])
                return aggT

            def c_side(layer):
                src = ext["featC_cc"] if layer == 1 else ag_out["cc"]
                wkey = "w1cc" if layer == 1 else "w2cc"
                for ch in meta["cc"]["chunks"]:
                    m = fetch_chunk("cc", ch, src)
                    for w in range(ch["w0"], ch["w1"]):
                        rows = LAST_ROWS if w == NW - 1 else P
                        aggT = window_agg("cc", ch, w, m, sb["ndcc_b"], "pA")
                        pO = psum.tile([P, P], f32, tag="pO", name=f"pOc_{layer}_{w}")
                        nc.tensor.matmul(pO[:], lhsT=aggT[:], rhs=sb[wkey][:],
                                         start=True, stop=True)
                        if layer == 1:
                            hC = work.tile([P, P], bf, tag="hC", name=f"hC_{w}")
                            nc.vector.tensor_add(hC[:], pO[:], sb["b1C_b"][:])
                            nc.vector.tensor_tensor(out=hC[:], in0=hC[:],
                                                    in1=sb["zeros_b"][:], op=AOP.max)
                            hCcc = work.tile([P, P], bf, tag="hCcc", name=f"hCcc_{w}")
                            nc.vector.tensor_tensor(
                                out=hCcc[:], in0=hC[:],
                                in1=col_bcast(sb["nsrccc_p"], w), op=AOP.mult)
                            hCcn = work.tile([P, P], bf, tag="hCcn", name=f"hCcn_{w}")
                            nc.vector.tensor_tensor(
                                out=hCcn[:], in0=hC[:],
                                in1=col_bcast(sb["nsrccn_p"], w), op=AOP.mult)
                            nc.sync.dma_start(out=ag_in["cc"][w * P:w * P + rows, :],
                                              in_=hCcc[:rows, :])
                            nc.sync.dma_start(out=ag_in["cn"][w * P:w * P + rows, :],
                                              in_=hCcn[:rows, :])
                        else:
                            oC = work.tile([P, P], f32, tag="oC", name=f"oC_{w}")
                            nc.vector.tensor_add(oC[:], pO[:], sb["b2C_b"][:])
                            nc.sync.dma_start(out=oC_s[w * P:w * P + rows, :],
                                              in_=oC[:rows, :])

            def n_side(layer):
                src_cn = ext["featC_cn"] if layer == 1 else ag_out["cn"]
                src_nn = ext["featN_b"] if layer == 1 else ag_out["nn"]
                selfT = sb["featNT_s"] if layer == 1 else hNT
                wcn = sb["w1cn" if layer == 1 else "w2cn"]
                wng = sb["w1neigh" if layer == 1 else "w2neigh"]
                wsf = sb["w1self" if layer == 1 else "w2self"]
                ch_cn = meta["cn"]["chunks"]
                ch_nn = meta["nn"]["chunks"]
                assert len(ch_cn) == len(ch_nn)
                for ci in range(len(ch_cn)):
                    chc, chn = ch_cn[ci], ch_nn[ci]
                    m_cn = fetch_chunk("cn", chc, src_cn)
                    m_nn = fetch_chunk("nn", chn, src_nn)
                    for w in range(chc["w0"], chc["w1"]):
                        rows = LAST_ROWS if w == NW - 1 else P
                        aggTcn = window_agg("cn", chc, w, m_cn, sb["ndcn_b"], "pA")
                        aggTnn = window_agg("nn", chn, w, m_nn, sb["invd_b"], "pB")
                        pO = psum.tile([P, P], f32, tag="pO", name=f"pOn_{layer}_{w}")
                        nc.tensor.matmul(pO[:], lhsT=aggTcn[:], rhs=wcn[:],
                                         start=True, stop=False)
                        nc.tensor.matmul(pO[:], lhsT=aggTnn[:], rhs=wng[:],
                                         start=False, stop=False)
                        nc.tensor.matmul(pO[:], lhsT=selfT[:, w * P:w * P + P],
                                         rhs=wsf[:], start=False, stop=True)
                        if layer == 1:
                            hN = work.tile([P, P], bf, tag="hN", name=f"hN_{w}")
                            nc.vector.tensor_add(hN[:], pO[:], sb["b1N_b"][:])
                            nc.vector.tensor_tensor(out=hN[:], in0=hN[:],
                                                    in1=sb["zeros_b"][:], op=AOP.max)
                            nc.sync.dma_start(out=ag_in["nn"][w * P:w * P + rows, :],
                                              in_=hN[:rows, :])
                            pOT = psum.tile([P, P], f32, tag="pOT", name=f"pOT_{w}")
                            nc.tensor.matmul(pOT[:], lhsT=wcn[:], rhs=aggTcn[:],
                                             start=True, stop=False)
                            nc.tensor.matmul(pOT[:], lhsT=wng[:], rhs=aggTnn[:],
                                             start=False, stop=False)
                            nc.tensor.matmul(pOT[:], lhsT=wsf[:],
                                             rhs=selfT[:, w * P:w * P + P],
                                             start=False, stop=True)
                            nc.scalar.activation(
                                out=hNT[:, w * P:w * P + P], in_=pOT[:],
                                func=mybir.ActivationFunctionType.Relu,
                                bias=sb["b1N_col"][:, 0:1], scale=1.0)
                        else:
                            oN = work.tile([P, P], f32, tag="oN", name=f"oN_{w}")
                            nc.vector.tensor_add(oN[:], pO[:], sb["b2N_b"][:])
                            nc.sync.dma_start(out=oN_s[w * P:w * P + rows, :],
                                              in_=oN[:rows, :])

            c_side(1)
            for r in ("cc", "cn"):
                nc.gpsimd.collective_compute(
                    "AllGather", mybir.AluOpType.bypass,
                    replica_groups=[list(range(NCORES))],
                    ins=[ag_in[r].ap().opt()], outs=[ag_out[r].ap().opt()])
            n_side(1)
            nc.gpsimd.collective_compute(
                "AllGather", mybir.AluOpType.bypass,
                replica_groups=[list(range(NCORES))],
                ins=[ag_in["nn"].ap().opt()], outs=[ag_out["nn"].ap().opt()])
            c_side(2)
            n_side(2)

    nc.compile()
    return nc


# ----------------------------------------------------------------------------
# Entry point
# ----------------------------------------------------------------------------

def _fingerprint(inp):
    parts = []
    for k in sorted(inp):
        a = np.asarray(inp[k])
        parts.append((k, a.shape, str(a.dtype),
                      a.reshape(-1)[:8].tobytes() if a.size else b""))
    return hash(tuple(parts))


def _kernel_bass(_trace=False, **inputs):
    fp = _fingerprint(inputs)
    if fp not in _cache:
        in_maps, meta = _prep(inputs)
        nc = _build(meta)
        _cache[fp] = (nc, in_maps)
    nc, in_maps = _cache[fp]

    from concourse.bass_utils import run_bass_kernel_spmd
    res = run_bass_kernel_spmd(nc, in_maps, core_ids=list(range(NCORES)),
                               trace=_trace)
    _kernel_bass.last_results = res
    _kernel_bass.last_exec_time_ns = res.exec_time_ns

    oC = np.concatenate([res.results[c]["oC_s"] for c in range(NCORES)], axis=0)
    oN = np.concatenate([res.results[c]["oN_s"] for c in range(NCORES)], axis=0)
    return oC.astype(np.float32), oN.astype(np.float32)


# ----------------------------------------------------------------------------
# NumPy fallback (correct but slow) in case the Bass path is unavailable
# ----------------------------------------------------------------------------

def _kernel_numpy(feat_C, feat_N, W1_cc, b1_cc, W1_cn, b1_cn, W1_self,
                  W1_neigh, b1_nn, W2_cc, b2_cc, W2_cn, b2_cn, W2_self,
                  W2_neigh, b2_nn, cc_src, cc_dst, cn_src, cn_dst, nn_src,
                  nn_dst):
    from scipy import sparse as sp

    def deg(x):
        return np.bincount(np.asarray(x, np.int64), minlength=N).astype(np.float32)

    def gcn(x, src, dst, W, b):
        ns = np.maximum(deg(src), 1.0) ** -0.5
        nd = np.maximum(deg(dst), 1.0) ** -0.5
        w = (nd[dst] * ns[src]).astype(np.float32)
        A = sp.csr_matrix((w, (dst, src)), shape=(N, N), dtype=np.float32)
        return (A @ x) @ W + b

    def sage(x, src, dst, Ws, Wn, b):
        di = np.maximum(deg(dst), 1.0)
        A = sp.csr_matrix(((1.0 / di)[dst].astype(np.float32), (dst, src)),
                          shape=(N, N), dtype=np.float32)
        return x @ Ws + (A @ x) @ Wn + b

    feat_C = np.asarray(feat_C, np.float32)
    feat_N = np.asarray(feat_N, np.float32)
    hC = np.maximum(gcn(feat_C, cc_src, cc_dst, W1_cc, b1_cc), 0.0)
    hN = np.maximum(gcn(feat_C, cn_src, cn_dst, W1_cn, b1_cn)
                    + sage(feat_N, nn_src, nn_dst, W1_self, W1_neigh, b1_nn), 0.0)
    oC = gcn(hC, cc_src, cc_dst, W2_cc, b2_cc)
    oN = (gcn(hC, cn_src, cn_dst, W2_cn, b2_cn)
          + sage(hN, nn_src, nn_dst, W2_self, W2_neigh, b2_nn))
    return oC.astype(np.float32), oN.astype(np.float32)


def kernel(_trace=False, **inputs):
    try:
        return _kernel_bass(_trace=_trace, **inputs)
    except Exception:
        import traceback
        traceback.print_exc()
        return _kernel_numpy(**inputs)


# revision 3
# speedup vs baseline: 1.0008x; 1.0008x over previous
"""HGNN 2-layer hetero GNN on 8 TRN2 NeuronCores via Bass/Tile — v2 hybrid.

v2 vs baseline:
  - Gather calls batched across windows: per relation the tile stream is laid
    out as chunks of consecutive windows, each chunk split into a lo-block
    (src < 32768) and hi-block so one dma_gather call covers many windows.
  - Each block's leading fraction F_IND of tiles is fetched with
    indirect_dma_start (async, ~0 Pool engine time) while dma_gather
    (Pool-bound, ~961ns/tile) covers the rest -> the two mechanisms overlap.
  - AllGathers issued as early as possible (cc/cn after c-side layer 1,
    nn after n-side layer 1) so they hide under gather work.
  - Slow tensor_scalar DVE epilogues replaced with tensor_tensor forms.
"""
import math
import numpy as np
import ml_dtypes

bf16 = ml_dtypes.bfloat16

N = 50000
D = 128
P = 128
NCORES = 8
ROWS = N // NCORES          # 6250 dst rows per core
NW = math.ceil(ROWS / P)    # 49 windows (48*128 + 106)
LAST_ROWS = ROWS - (NW - 1) * P  # 106
ROWSP = NW * P              # 6272
PAD_DOFF = 200.0            # never matches iota 0..127
HI = 32768                  # int16 index split point

CH_C = 6                    # windows per chunk, cc relation
CH_N = 4                    # windows per chunk, cn/nn relations
import os
F_IND = float(os.environ.get("K2_F_IND", "0.0"))                 # fraction of tiles per block routed to indirect
MAX_CALL_TILES = 32         # max tiles per dma_gather / indirect call

_cache = {}


# ----------------------------------------------------------------------------
# Host preprocessing
# ----------------------------------------------------------------------------

def _pack_relation(src, dst, chunk_windows):
    """Dense chunked lo/hi block packing.

    Chunks of `chunk_windows` windows. Within a chunk: a lo block then a hi
    block, each a dense slot stream (per-window slot range = max-over-cores
    count, no per-window tile alignment), padded to a 128 multiple at block
    end. doff is the chunk-local dst offset (f32, exact), compared against a
    chunk-wide iota, so tiles may straddle windows.
    """
    src = np.asarray(src, dtype=np.int64)
    dst = np.asarray(dst, dtype=np.int64)
    is_hi = (src >= HI).astype(np.int64)
    core = dst // ROWS
    dloc = dst % ROWS
    w = dloc // P
    gid2 = (core * NW + w) * 2 + is_hi
    order = np.argsort(gid2, kind="stable")
    src_s = src[order]
    gid2_s = gid2[order]
    dloc_s = dloc[order]
    core_s = core[order]
    w_s = w[order]
    hi_s = is_hi[order]

    counts = np.bincount(gid2_s, minlength=NCORES * NW * 2)
    starts = np.concatenate([[0], np.cumsum(counts)[:-1]])
    rank = np.arange(len(src_s)) - starts[gid2_s]

    cw = counts.reshape(NCORES, NW, 2)
    mx = cw.max(axis=0)                      # [NW, 2] max count over cores

    chunks = []
    wmeta = [None] * NW     # per window: spans in global tile coords
    slot0 = np.zeros((NW, 2), np.int64)      # global slot of each group
    S = 0                   # global tiles so far
    w0 = 0
    while w0 < NW:
        w1 = min(w0 + chunk_windows, NW)
        base = S
        # lo block
        off = 0
        for wi in range(w0, w1):
            slot0[wi, 0] = base * P + off
            off += int(mx[wi, 0])
        lo_T = -(-off // P)
        # hi block
        off = 0
        for wi in range(w0, w1):
            slot0[wi, 1] = (base + lo_T) * P + off
            off += int(mx[wi, 1])
        hi_T = -(-off // P)
        tot_T = lo_T + hi_T
        for wi in range(w0, w1):
            lo_g0 = int(slot0[wi, 0] // P)
            lo_g1 = int(-(-(slot0[wi, 0] + mx[wi, 0]) // P))
            hi_g0 = int(slot0[wi, 1] // P)
            hi_g1 = int(-(-(slot0[wi, 1] + mx[wi, 1]) // P))
            wmeta[wi] = dict(lo_g0=lo_g0, lo_span=lo_g1 - lo_g0,
                             hi_g0=hi_g0, hi_span=hi_g1 - hi_g0,
                             iota_off=(wi - w0) * P)
        chunks.append(dict(w0=w0, w1=w1, base=base, lo_T=lo_T,
                           hi_T=hi_T, tot_T=tot_T))
        S += tot_T
        w0 = w1

    slot = slot0[w_s, hi_s] + rank
    colG = slot // P
    pG = slot % P

    # chunk-local dst offset (0 .. chunk_windows*128), pads = -1000
    wchunk0 = (w_s // chunk_windows) * chunk_windows
    doff = np.full((NCORES, P, S), -1000.0, dtype=np.float32)
    doff[core_s, pG, colG] = (dloc_s - wchunk0 * P).astype(np.float32)

    idx16 = np.zeros((NCORES, 16, S * 8), dtype=np.int16)
    col16 = colG * 8 + (pG // 16)
    row16 = pG % 16
    val16 = np.where(hi_s == 1, src_s - HI, src_s).astype(np.int16)
    idx16[core_s, row16, col16] = val16
    idx16_full = np.ascontiguousarray(np.tile(idx16, (1, 8, 1)))

    idx32 = np.zeros((NCORES, P, S), dtype=np.int32)
    idx32[core_s, pG, colG] = src_s.astype(np.int32)

    return dict(S=S, chunks=chunks, wmeta=wmeta,
                idx16=idx16_full, idx32=idx32, doff=doff)
def _bcast_rows(vec):
    v = np.zeros(ROWSP, dtype=vec.dtype)
    v[:ROWS] = vec
    return np.ascontiguousarray(np.broadcast_to(v[None, :], (P, ROWSP)))


def _pack_part(vec):
    out = np.zeros((P, NW), dtype=np.float32)
    padded = np.zeros(NW * P, dtype=np.float32)
    padded[:ROWS] = vec
    out[:] = padded.reshape(NW, P).T
    return out


def _prep(inp):
    feat_C = np.asarray(inp["feat_C"], dtype=np.float32)
    feat_N = np.asarray(inp["feat_N"], dtype=np.float32)

    def deg(x, n):
        return np.bincount(np.asarray(x, dtype=np.int64), minlength=n).astype(np.float32)

    cc_src = np.asarray(inp["cc_src"]); cc_dst = np.asarray(inp["cc_dst"])
    cn_src = np.asarray(inp["cn_src"]); cn_dst = np.asarray(inp["cn_dst"])
    nn_src = np.asarray(inp["nn_src"]); nn_dst = np.asarray(inp["nn_dst"])

    ns_cc = np.maximum(deg(cc_src, N), 1.0) ** -0.5
    nd_cc = np.maximum(deg(cc_dst, N), 1.0) ** -0.5
    ns_cn = np.maximum(deg(cn_src, N), 1.0) ** -0.5
    nd_cn = np.maximum(deg(cn_dst, N), 1.0) ** -0.5
    invd_nn = 1.0 / np.maximum(deg(nn_dst, N), 1.0)

    featC_cc = (feat_C * ns_cc[:, None]).astype(bf16)
    featC_cn = (feat_C * ns_cn[:, None]).astype(bf16)
    featN_b = feat_N.astype(bf16)

    rel_cc = _pack_relation(cc_src, cc_dst, CH_C)
    rel_cn = _pack_relation(cn_src, cn_dst, CH_N)
    rel_nn = _pack_relation(nn_src, nn_dst, CH_N)

    iota_cw = np.ascontiguousarray(np.broadcast_to(
        np.arange(CH_C * P, dtype=np.float32)[None, :], (P, CH_C * P)))

    def bb(v):
        return np.ascontiguousarray(np.broadcast_to(
            np.asarray(v, np.float32)[None, :], (P, P)))

    b1N = np.asarray(inp["b1_cn"], np.float32) + np.asarray(inp["b1_nn"], np.float32)
    b2N = np.asarray(inp["b2_cn"], np.float32) + np.asarray(inp["b2_nn"], np.float32)

    Wn = ["w1cc", "w1cn", "w1self", "w1neigh", "w2cc", "w2cn", "w2self", "w2neigh"]
    Wv = [inp["W1_cc"], inp["W1_cn"], inp["W1_self"], inp["W1_neigh"],
          inp["W2_cc"], inp["W2_cn"], inp["W2_self"], inp["W2_neigh"]]

    in_maps = []
    for c in range(NCORES):
        r0, r1 = c * ROWS, (c + 1) * ROWS
        m = {
            "featC_cc": featC_cc,
            "featC_cn": featC_cn,
            "featN_b": featN_b,
            "featNT_s": np.ascontiguousarray(np.concatenate(
                [featN_b[r0:r1], np.zeros((ROWSP - ROWS, D), bf16)]).T),
            "ndcc_b": _bcast_rows(nd_cc[r0:r1]).astype(bf16),
            "ndcn_b": _bcast_rows(nd_cn[r0:r1]).astype(bf16),
            "invd_b": _bcast_rows(invd_nn[r0:r1]).astype(bf16),
            "nsrccc_p": _pack_part(ns_cc[r0:r1]),
            "nsrccn_p": _pack_part(ns_cn[r0:r1]),
            "iota_cw": iota_cw,
            "zeros_b": np.zeros((P, P), np.float32),
            "b1C_b": bb(inp["b1_cc"]), "b1N_b": bb(b1N),
            "b2C_b": bb(inp["b2_cc"]), "b2N_b": bb(b2N),
            "b1N_col": np.ascontiguousarray(b1N[:, None]),
        }
        for rel, dat in (("cc", rel_cc), ("cn", rel_cn), ("nn", rel_nn)):
            m[f"idx_{rel}"] = dat["idx16"][c]
            m[f"idx32_{rel}"] = dat["idx32"][c]
            m[f"doff_{rel}"] = dat["doff"][c]
        for nm, v in zip(Wn, Wv):
            m[nm] = np.asarray(v, np.float32).astype(bf16)
        in_maps.append(m)

    meta = {}
    for rel, dat in (("cc", rel_cc), ("cn", rel_cn), ("nn", rel_nn)):
        meta[rel] = {k: dat[k] for k in ("S", "chunks", "wmeta")}
    return in_maps, meta


# ----------------------------------------------------------------------------
# Bass kernel builder
# ----------------------------------------------------------------------------

def _build(meta):
    import concourse.bass as bass
    import concourse.bacc as bacc
    import concourse.mybir as mybir
    import concourse.tile as tile

    f32 = mybir.dt.float32
    bf = mybir.dt.bfloat16
    i16 = mybir.dt.int16
    i32 = mybir.dt.int32
    AOP = mybir.AluOpType

    nc = bacc.Bacc(None, target_bir_lowering=False)

    ext = {}
    def din(name, shape, dtype):
        ext[name] = nc.dram_tensor(name, shape, dtype, kind="ExternalInput")
        return ext[name]

    din("featC_cc", [N, D], bf)
    din("featC_cn", [N, D], bf)
    din("featN_b", [N, D], bf)
    din("featNT_s", [P, ROWSP], bf)
    for rel in ("cc", "cn", "nn"):
        S = meta[rel]["S"]
        din(f"idx_{rel}", [P, S * 8], i16)
        din(f"idx32_{rel}", [P, S], i32)
        din(f"doff_{rel}", [P, S], f32)
    din("ndcc_b", [P, ROWSP], bf)
    din("ndcn_b", [P, ROWSP], bf)
    din("invd_b", [P, ROWSP], bf)
    din("nsrccc_p", [P, NW], f32)
    din("nsrccn_p", [P, NW], f32)
    din("iota_cw", [P, CH_C * P], f32)
    din("zeros_b", [P, P], f32)
    din("b1C_b", [P, P], f32)
    din("b1N_b", [P, P], f32)
    din("b2C_b", [P, P], f32)
    din("b2N_b", [P, P], f32)
    din("b1N_col", [P, 1], f32)
    for nm in ("w1cc", "w1cn", "w1self", "w1neigh", "w2cc", "w2cn", "w2self", "w2neigh"):
        din(nm, [D, D], bf)

    oC_s = nc.dram_tensor("oC_s", [ROWS, D], f32, kind="ExternalOutput")
    oN_s = nc.dram_tensor("oN_s", [ROWS, D], f32, kind="ExternalOutput")

    ag_in = {r: nc.dram_tensor(f"agin_{r}", [ROWS, D], bf) for r in ("cc", "cn", "nn")}
    ag_out = {r: nc.dram_tensor(f"agout_{r}", [N, D], bf, addr_space="Shared")
              for r in ("cc", "cn", "nn")}

    with tile.TileContext(nc) as tc:
        import contextlib
        with contextlib.ExitStack() as ctx:
            cpool = ctx.enter_context(tc.tile_pool(name="consts", bufs=1))
            mpool = ctx.enter_context(tc.tile_pool(name="mchunks", bufs=2))
            work = ctx.enter_context(tc.tile_pool(name="work", bufs=3))
            psum = ctx.enter_context(tc.tile_pool(name="psum", bufs=2, space="PSUM"))

            sb = {}
            def load(name, shape, dtype):
                t = cpool.tile(shape, dtype, name=f"sb_{name}")
                nc.sync.dma_start(out=t[:], in_=ext[name][:])
                sb[name] = t
                return t

            for rel in ("cc", "cn", "nn"):
                S = meta[rel]["S"]
                load(f"idx_{rel}", [P, S * 8], i16)
                load(f"idx32_{rel}", [P, S], i32)
                load(f"doff_{rel}", [P, S], f32)
            load("ndcc_b", [P, ROWSP], bf)
            load("ndcn_b", [P, ROWSP], bf)
            load("invd_b", [P, ROWSP], bf)
            load("nsrccc_p", [P, NW], f32)
            load("nsrccn_p", [P, NW], f32)
            load("iota_cw", [P, CH_C * P], f32)
            load("zeros_b", [P, P], f32)
            load("b1C_b", [P, P], f32)
            load("b1N_b", [P, P], f32)
            load("b2C_b", [P, P], f32)
            load("b2N_b", [P, P], f32)
            load("b1N_col", [P, 1], f32)
            load("featNT_s", [P, ROWSP], bf)
            for nm in ("w1cc", "w1cn", "w1self", "w1neigh",
                       "w2cc", "w2cn", "w2self", "w2neigh"):
                load(nm, [D, D], bf)

            hNT = cpool.tile([P, ROWSP], bf, name="hNT")

            def col_bcast(t, c):
                """[128,1] column c of tile t -> [128, P] broadcast AP."""
                col = t[:, c:c + 1]
                return bass.AP(col.tensor, col.offset, [col.ap[0], [0, P]])

            def fetch_chunk(rel, ch, srcs):
                """Gather one chunk's tiles into a fresh m buffer.

                Leading F_IND of each (lo, hi) block via indirect_dma_start,
                remainder via dma_gather (lo/hi idx16 with split tables).
                Returns the m tile [P, tot_T, P].
                """
                md = meta[rel]
                tot = ch["tot_T"]
                base = ch["base"]
                m = mpool.tile([P, tot, P], bf, tag=f"m_{rel}", name=f"m_{rel}_{ch['w0']}")
                idx32 = sb[f"idx32_{rel}"]
                idx16 = sb[f"idx_{rel}"]

                for half, blk_off, blk_T in ((0, 0, ch["lo_T"]),
                                             (1, ch["lo_T"], ch["hi_T"])):
                    if blk_T == 0:
                        continue
                    a = int(round(blk_T * F_IND))
                    # indirect leading part: HW supports one row per partition
                    # per call, so issue one call per 128-row tile (~30ns each
                    # on the Pool sequencer; DGE + transfer run async).
                    for t0 in range(a):
                        g0 = base + blk_off + t0
                        nc.gpsimd.indirect_dma_start(
                            out=m[:, blk_off + t0, :],
                            out_offset=None,
                            in_=srcs[:, :],
                            in_offset=bass.IndirectOffsetOnAxis(
                                ap=idx32[:, g0:g0 + 1], axis=0))
                    # gather rest
                    t0 = a
                    tab = srcs[:, :] if half == 0 else srcs[HI:, :]
                    while t0 < blk_T:
                        n = min(MAX_CALL_TILES, blk_T - t0)
                        g0 = base + blk_off + t0
                        nc.gpsimd.dma_gather(
                            m[:, blk_off + t0:blk_off + t0 + n, :], tab,
                            idx16[:, g0 * 8:(g0 + n) * 8],
                            n * P, n * P, P, single_packet=False)
                        t0 += n
                return m

            def window_agg(rel, ch, w, m, norm_sb, ptag):
                """One window's segment aggregate -> aggT [P(feat), P(dst)]."""
                wm = meta[rel]["wmeta"][w]
                lo_span, hi_span = wm["lo_span"], wm["hi_span"]
                T = lo_span + hi_span
                base = ch["base"]
                doff = sb[f"doff_{rel}"]
                io = sb["iota_cw"][:, wm["iota_off"]:wm["iota_off"] + P]

                O = work.tile([P, T * P], bf, tag="O", name=f"O_{rel}_{w}")
                if lo_span:
                    d = doff[:, wm["lo_g0"]:wm["lo_g0"] + lo_span]
                    in0 = bass.AP(d.tensor, d.offset, d.ap + [[0, P]])
                    in1 = bass.AP(io.tensor, io.offset,
                                  [io.ap[0], [0, lo_span], io.ap[1]])
                    nc.vector.tensor_tensor(out=O[:, 0:lo_span * P], in0=in0,
                                            in1=in1, op=AOP.is_equal)
                if hi_span:
                    d = doff[:, wm["hi_g0"]:wm["hi_g0"] + hi_span]
                    in0 = bass.AP(d.tensor, d.offset, d.ap + [[0, P]])
                    in1 = bass.AP(io.tensor, io.offset,
                                  [io.ap[0], [0, hi_span], io.ap[1]])
                    nc.vector.tensor_tensor(out=O[:, lo_span * P:T * P], in0=in0,
                                            in1=in1, op=AOP.is_equal)

                pA = psum.tile([P, P], f32, tag=ptag, name=f"pA_{rel}_{w}")
                k = 0
                for t in range(lo_span):
                    nc.tensor.matmul(pA[:], lhsT=m[:, wm["lo_g0"] - base + t, :],
                                     rhs=O[:, k * P:(k + 1) * P],
                                     start=(k == 0), stop=(k == T - 1))
                    k += 1
                for t in range(hi_span):
                    nc.tensor.matmul(pA[:], lhsT=m[:, wm["hi_g0"] - base + t, :],
                                     rhs=O[:, k * P:(k + 1) * P],
                                     start=(k == 0), stop=(k == T - 1))
                    k += 1
                aggT = work.tile([P, P], bf, tag=f"aggT_{rel}", name=f"aggT_{rel}_{w}")
                nc.vector.tensor_mul(aggT[:], pA[:], norm_sb[:, w * P:w * P + P])
                return aggT

            def c_side(layer):
                src = ext["featC_cc"] if layer == 1 else ag_out["cc"]
                wkey = "w1cc" if layer == 1 else "w2cc"
                for ch in meta["cc"]["chunks"]:
                    m = fetch_chunk("cc", ch, src)
                    for w in range(ch["w0"], ch["w1"]):
                        rows = LAST_ROWS if w == NW - 1 else P
                        aggT = window_agg("cc", ch, w, m, sb["ndcc_b"], "pA")
                        pO = psum.tile([P, P], f32, tag="pO", name=f"pOc_{layer}_{w}")
                        nc.tensor.matmul(pO[:], lhsT=aggT[:], rhs=sb[wkey][:],
                                         start=True, stop=True)
                        if layer == 1:
                            hC = work.tile([P, P], bf, tag="hC", name=f"hC_{w}")
                            nc.vector.tensor_add(hC[:], pO[:], sb["b1C_b"][:])
                            nc.vector.tensor_tensor(out=hC[:], in0=hC[:],
                                                    in1=sb["zeros_b"][:], op=AOP.max)
                            hCcc = work.tile([P, P], bf, tag="hCcc", name=f"hCcc_{w}")
                            nc.vector.tensor_tensor(
                                out=hCcc[:], in0=hC[:],
                                in1=col_bcast(sb["nsrccc_p"], w), op=AOP.mult)
                            hCcn = work.tile([P, P], bf, tag="hCcn", name=f"hCcn_{w}")
                            nc.vector.tensor_tensor(
                                out=hCcn[:], in0=hC[:],
                                in1=col_bcast(sb["nsrccn_p"], w), op=AOP.mult)
                            nc.sync.dma_start(out=ag_in["cc"][w * P:w * P + rows, :],
                                              in_=hCcc[:rows, :])
                            nc.sync.dma_start(out=ag_in["cn"][w * P:w * P + rows, :],
                                              in_=hCcn[:rows, :])
                        else:
                            oC = work.tile([P, P], f32, tag="oC", name=f"oC_{w}")
                            nc.vector.tensor_add(oC[:], pO[:], sb["b2C_b"][:])
                            nc.sync.dma_start(out=oC_s[w * P:w * P + rows, :],
                                              in_=oC[:rows, :])

            def n_side(layer):
                src_cn = ext["featC_cn"] if layer == 1 else ag_out["cn"]
                src_nn = ext["featN_b"] if layer == 1 else ag_out["nn"]
                selfT = sb["featNT_s"] if layer == 1 else hNT
                wcn = sb["w1cn" if layer == 1 else "w2cn"]
                wng = sb["w1neigh" if layer == 1 else "w2neigh"]
                wsf = sb["w1self" if layer == 1 else "w2self"]
                ch_cn = meta["cn"]["chunks"]
                ch_nn = meta["nn"]["chunks"]
                assert len(ch_cn) == len(ch_nn)
                for ci in range(len(ch_cn)):
                    chc, chn = ch_cn[ci], ch_nn[ci]
                    m_cn = fetch_chunk("cn", chc, src_cn)
                    m_nn = fetch_chunk("nn", chn, src_nn)
                    for w in range(chc["w0"], chc["w1"]):
                        rows = LAST_ROWS if w == NW - 1 else P
                        aggTcn = window_agg("cn", chc, w, m_cn, sb["ndcn_b"], "pA")
                        aggTnn = window_agg("nn", chn, w, m_nn, sb["invd_b"], "pB")
                        pO = psum.tile([P, P], f32, tag="pO", name=f"pOn_{layer}_{w}")
                        nc.tensor.matmul(pO[:], lhsT=aggTcn[:], rhs=wcn[:],
                                         start=True, stop=False)
                        nc.tensor.matmul(pO[:], lhsT=aggTnn[:], rhs=wng[:],
                                         start=False, stop=False)
                        nc.tensor.matmul(pO[:], lhsT=selfT[:, w * P:w * P + P],
                                         rhs=wsf[:], start=False, stop=True)
                        if layer == 1:
                            hN = work.tile([P, P], bf, tag="hN", name=f"hN_{w}")
                            nc.vector.tensor_add(hN[:], pO[:], sb["b1N_b"][:])
                            nc.vector.tensor_tensor(out=hN[:], in0=hN[:],
                                                    in1=sb["zeros_b"][:], op=AOP.max)
                            nc.sync.dma_start(out=ag_in["nn"][w * P:w * P + rows, :],
                                              in_=hN[:rows, :])
                            pOT = psum.tile([P, P], f32, tag="pOT", name=f"pOT_{w}")
                            nc.tensor.matmul(pOT[:], lhsT=wcn[:], rhs=aggTcn[:],
                                             start=True, stop=False)
                            nc.tensor.matmul(pOT[:], lhsT=wng[:], rhs=aggTnn[:],
                                             start=False, stop=False)
                            nc.tensor.matmul(pOT[:], lhsT=wsf[:],
                                             rhs=selfT[:, w * P:w * P + P],
                                             start=False, stop=True)
                            nc.scalar.activation(
                                out=hNT[:, w * P:w * P + P], in_=pOT[:],
                                func=mybir.ActivationFunctionType.Relu,
                                bias=sb["b1N_col"][:, 0:1], scale=1.0)
                        else:
                            oN = work.tile([P, P], f32, tag="oN", name=f"oN_{w}")
                            nc.vector.tensor_add(oN[:], pO[:], sb["b2N_b"][:])
                            nc.sync.dma_start(out=oN_s[w * P:w * P + rows, :],
                                              in_=oN[:rows, :])

            c_side(1)
            for r in ("cc", "cn"):
                nc.gpsimd.collective_compute(
                    "AllGather", mybir.AluOpType.bypass,
                    replica_groups=[list(range(NCORES))],
                    ins=[ag_in[r].ap().opt()], outs=[ag_out[r].ap().opt()])
            n_side(1)
            nc.gpsimd.collective_compute(
                "AllGather", mybir.AluOpType.bypass,
                replica_groups=[list(range(NCORES))],
                ins=[ag_in["nn"].ap().opt()], outs=[ag_out["nn"].ap().opt()])
            c_side(2)
            n_side(2)

    nc.compile()
    return nc


# ----------------------------------------------------------------------------
# Entry point
# ----------------------------------------------------------------------------

def _fingerprint(inp):
    parts = []
    for k in sorted(inp):
        a = np.asarray(inp[k])
        parts.append((k, a.shape, str(a.dtype),
                      a.reshape(-1)[:8].tobytes() if a.size else b""))
    return hash(tuple(parts))


def _kernel_bass(_trace=False, **inputs):
    fp = _fingerprint(inputs)
    if fp not in _cache:
        in_maps, meta = _prep(inputs)
        nc = _build(meta)
        _cache[fp] = (nc, in_maps)
    nc, in_maps = _cache[fp]

    from concourse.bass_utils import run_bass_kernel_spmd
    res = run_bass_kernel_spmd(nc, in_maps, core_ids=list(range(NCORES)),
                               trace=_trace)
    _kernel_bass.last_results = res
    _kernel_bass.last_exec_time_ns = res.exec_time_ns

    oC = np.concatenate([res.results[c]["oC_s"] for c in range(NCORES)], axis=0)
    oN = np.concatenate([res.results[c]["oN_s"] for c in range(NCORES)], axis=0)
    return oC.astype(np.float32), oN.astype(np.float32)


# ----------------------------------------------------------------------------
# NumPy fallback (correct but slow) in case the Bass path is unavailable
# ----------------------------------------------------------------------------

def _kernel_numpy(feat_C, feat_N, W1_cc, b1_cc, W1_cn, b1_cn, W1_self,
                  W1_neigh, b1_nn, W2_cc, b2_cc, W2_cn, b2_cn, W2_self,
                  W2_neigh, b2_nn, cc_src, cc_dst, cn_src, cn_dst, nn_src,
                  nn_dst):
    from scipy import sparse as sp

    def deg(x):
        return np.bincount(np.asarray(x, np.int64), minlength=N).astype(np.float32)

    def gcn(x, src, dst, W, b):
        ns = np.maximum(deg(src), 1.0) ** -0.5
        nd = np.maximum(deg(dst), 1.0) ** -0.5
        w = (nd[dst] * ns[src]).astype(np.float32)
        A = sp.csr_matrix((w, (dst, src)), shape=(N, N), dtype=np.float32)
        return (A @ x) @ W + b

    def sage(x, src, dst, Ws, Wn, b):
        di = np.maximum(deg(dst), 1.0)
        A = sp.csr_matrix(((1.0 / di)[dst].astype(np.float32), (dst, src)),
                          shape=(N, N), dtype=np.float32)
        return x @ Ws + (A @ x) @ Wn + b

    feat_C = np.asarray(feat_C, np.float32)
    feat_N = np.asarray(feat_N, np.float32)
    hC = np.maximum(gcn(feat_C, cc_src, cc_dst, W1_cc, b1_cc), 0.0)
    hN = np.maximum(gcn(feat_C, cn_src, cn_dst, W1_cn, b1_cn)
                    + sage(feat_N, nn_src, nn_dst, W1_self, W1_neigh, b1_nn), 0.0)
    oC = gcn(hC, cc_src, cc_dst, W2_cc, b2_cc)
    oN = (gcn(hC, cn_src, cn_dst, W2_cn, b2_cn)
          + sage(hN, nn_src, nn_dst, W2_self, W2_neigh, b2_nn))
    return oC.astype(np.float32), oN.astype(np.float32)


def kernel(_trace=False, **inputs):
    try:
        return _kernel_bass(_trace=_trace, **inputs)
    except Exception:
        import traceback
        traceback.print_exc()
        return _kernel_numpy(**inputs)
